# revision 36
# baseline (speedup 1.0000x reference)
"""Causal ReLU-attention block (qkv proj + per-head attention) on 8 trn2 cores.

Sharding: pure data-parallel over batch (B=8 -> 1 batch element per core).

Per-core structure — one fused PE stream so evictions (ACT/DVE) and DMA
overlap matmuls everywhere and the HAM clock gate never re-throttles:
  warm-up MMs (bridge the DMA-paced head, warm the clock) ->
  q/k projections for group 0 ->
  3 attention groups of 2 head-pair streams, software-pipelined:
    per step: score pairs (row-quadrant packed, K=64x2), lagged att@v
    pairs (col-quadrant packed, M=64x2), plus paced fillers = this
    group's v-projection slices + next group's q/k projections;
    group boundaries overlap (tail att@v of group g interleaves with
    first scores of g+1) to avoid PE dips.

PSUM: 2x[128,2,512] score tiles + 2x[128,512] filler tiles +
2x[128,512] y accumulators = 8 banks.  Scores evict via one
relu-activation per item alternating ACT/DVE (keeps att@v pairs in
sync); v adds its bias from an on-chip rank-1-broadcast tile.

DMA: host packs x/W/biases into per-partition-contiguous blocks in
consumption order; sync ring carries Wq01 + x-even + y out, scalar ring
carries biases + Wk01 + x-odd + remaining W blocks (issued between ACT
evictions).  Measured ~119-127us traced (run-to-run power-state
variance ~+-4us) vs 141us for the phase-sequential baseline.
"""

import sys
from collections import deque
from contextlib import ExitStack

sys.path.insert(0, "/opt/trn_rl_repo")

import ml_dtypes
import numpy as np

import concourse.bass as bass
import concourse.tile as tile
from concourse import bacc, bass_utils, mybir

P = 128
QW = 512  # t_q chunk width (PSUM bank = 512 fp32)

BF16 = mybir.dt.bfloat16
F32 = mybir.dt.float32
AF = mybir.ActivationFunctionType
ALU = mybir.AluOpType


def build_module(T=1024, C=768, H=12, n_cores=8):
    """Build + compile the per-core Bass module (same program on all cores)."""
    hd = C // H
    assert hd == 64 and H % 2 == 0 and C % P == 0 and T % QW == 0
    CT = C // P            # contraction tiles over C (6)
    TT = T // P            # t tiles (8)
    NQC = T // QW          # q chunks (2)
    NHP = H // 2           # head pairs (6)
    NG = NHP // 2          # attention groups of 2 head-pair streams (3)
    scale = 1.0 / float(np.sqrt(hd))

    nc = bacc.Bacc("TRN2", target_bir_lowering=False, debug=False,
                   num_devices=n_cores)

    # host-packed inputs: per-partition-contiguous blocks in use order
    #  xd: [p, 3 ct-pair blocks, 2 ct, T]
    #  wd: [p, 9 blocks, ct, 256] with blocks
    #      [q01, k01, v0, q23, k23, v1, q45, k45, v2]
    xd = nc.dram_tensor("xd", [P, CT, T], BF16, kind="ExternalInput").ap()
    wd = nc.dram_tensor("wd", [P, 9, CT, 256], BF16,
                        kind="ExternalInput").ap()
    bqk = nc.dram_tensor("bqk", [P, 2 * CT], F32, kind="ExternalInput").ap()
    bvr = nc.dram_tensor("bvr", [1, 2 * C], BF16, kind="ExternalInput").ap()
    yT = nc.dram_tensor("yT", [C, T], F32, kind="ExternalOutput").ap()

    with tile.TileContext(nc) as tc, ExitStack() as ctx:
        const = ctx.enter_context(tc.tile_pool(name="const", bufs=1))
        # PSUM: "s" 4 banks (score tiles), "f" 2 banks (qk/v chains),
        # "y" 2 banks (attention accumulators) = 8 banks total
        spsum = ctx.enter_context(tc.tile_pool(name="spsum", bufs=2, space="PSUM"))
        fpsum = ctx.enter_context(tc.tile_pool(name="fpsum", bufs=2, space="PSUM"))
        ypsum = ctx.enter_context(tc.tile_pool(name="ypsum", bufs=2, space="PSUM"))
        scb = ctx.enter_context(tc.tile_pool(name="scb", bufs=14))
        ysb = ctx.enter_context(tc.tile_pool(name="ysb", bufs=3))

        wt_sb = const.tile([P, 9, CT, 256], BF16)
        xt_sb = const.tile([P, CT, T], BF16)
        bqk_sb = const.tile([P, 2 * CT], F32)
        bvr_sb = const.tile([1, 2 * C], BF16)
        ones_sb = const.tile([1, P], BF16)
        bvf_sb = const.tile([P, 3, 2, 256], F32)  # bias bcast, doubled per slice
        qkT = const.tile([P, 2 * CT, T], BF16)   # o-tiles: q = 0..5, k = 6..11
        vsb = const.tile([P, TT, C], BF16)       # v in natural [t, o] layout
        mask_sb = const.tile([P, QW], BF16)

        # ---- input DMA issue (ring order == consumption order) ----
        # Split x per-ct across both rings; first W blocks lead on sync.
        # Remaining W blocks are interleaved between ACT evictions below.
        nc.sync.dma_start(wt_sb[:, 0], wd[:, 0])     # Wq01
        nc.scalar.dma_start(bqk_sb[:], bqk[:])
        nc.scalar.dma_start(bvr_sb[:], bvr[:])
        nc.scalar.dma_start(wt_sb[:, 1], wd[:, 1])   # Wk01
        for ct in range(CT):
            eng = nc.sync if ct % 2 == 0 else nc.scalar
            eng.dma_start(xt_sb[:, ct, :], xd[:, ct])
        w_dma = deque(range(2, 9))                   # v0, q23, k23, v1, ...

        def issue_w():
            if w_dma:
                b = w_dma.popleft()
                nc.scalar.dma_start(wt_sb[:, b], wd[:, b])

        # 0/1 upper-triangle mask const (also used as warm-up operand)
        nc.gpsimd.memset(ones_sb[:], 1.0)
        nc.gpsimd.memset(mask_sb[:], 1.0)
        nc.gpsimd.affine_select(
            mask_sb[:], mask_sb[:], pattern=[[1, QW]],
            compare_op=ALU.is_ge, fill=0.0, base=0, channel_multiplier=-1)

        # ---- PE warm-up: keep HAM busy while first inputs stream in ----
        warm_ps = ypsum.tile([P, QW], F32, tag="y", name="warm")
        for _ in range(14):
            nc.tensor.matmul(warm_ps[:], mask_sb[:, 0:P], mask_sb[:],
                             start=True, stop=True)

        # broadcast the v bias rows to all 128 partitions once (rank-1
        # matmuls), so v evictions can add them via plain tensor_tensor
        for s3 in range(3):
            bps = fpsum.tile([P, QW], F32, tag="f", name="bv_ps")
            nc.tensor.matmul(bps[:], ones_sb[:],
                             bvr_sb[:, s3 * QW:(s3 + 1) * QW],
                             start=True, stop=True)
            nc.vector.tensor_copy(
                bvf_sb[:, s3].rearrange("p a b -> p (a b)"), bps[:])

        evict = [0]

        def emit_qk(ot):
            # ot 0..5 = q features (head pair = ot), 6..11 = k features;
            # one 1-bank psum tile + one ACT bias-evict per q chunk
            j = ot if ot < CT else ot - CT
            blk = 3 * (j // 2) + (0 if ot < CT else 1)
            off = (j % 2) * P
            for qc in range(NQC):
                ps = fpsum.tile([P, QW], F32, tag="f", name="qk_ps")
                for ct in range(CT):
                    nc.tensor.matmul(
                        ps[:],
                        wt_sb[:, blk, ct, off:off + P],
                        xt_sb[:, ct, qc * QW:(qc + 1) * QW],
                        start=(ct == 0), stop=(ct == CT - 1),
                    )
                if qc == 0:
                    issue_w()
                nc.scalar.activation(
                    qkT[:, ot, qc * QW:(qc + 1) * QW], ps[:],
                    AF.Identity, bias=bqk_sb[:, ot:ot + 1])

        def emit_v(s, tp):
            # v features s*256..(s+1)*256 for t-tiles 2tp, 2tp+1; bias is
            # added at eviction from the broadcast bias tile
            ps = fpsum.tile([P, QW], F32, tag="f", name="v_ps")
            for j in range(2):
                tt = 2 * tp + j
                for ct in range(CT):
                    nc.tensor.matmul(
                        ps[:, j * 256:(j + 1) * 256],
                        xt_sb[:, ct, tt * P:(tt + 1) * P],
                        wt_sb[:, 3 * s + 2, ct, :],
                        start=(ct == 0), stop=(ct == CT - 1),
                    )
            dst = vsb[:, 2 * tp:2 * tp + 2, s * 256:(s + 1) * 256]
            src = ps.rearrange("p (a b) -> p a b", a=2)
            nc.vector.tensor_tensor(dst, src, bvf_sb[:, s], ALU.add)
            evict[0] += 1

        def relu_evict(dst, src):
            # relu(scale * s): PSUM -> SBUF bf16, alternating ACT / DVE per
            # item (one instruction per item keeps the av pair in sync)
            if evict[0] % 2 == 0:
                nc.scalar.activation(dst, src, AF.Relu, scale=scale)
            else:
                nc.vector.tensor_scalar(dst, src, scale, 0.0, ALU.mult, ALU.max)
            evict[0] += 1

        def attention_closures(hp):
            """Per-item (scores, att@v) emission closures for one head pair;
            interleaver runs att@v LAG items behind scores."""
            items = []
            for qc in range(NQC):
                kb_hi = min((qc * QW + QW - 1) // P, TT - 1)
                for kb in range(kb_hi + 1):
                    items.append((qc, kb, kb_hi))
            state = {"s": {}, "y": {}}
            sc_fns, av_fns = [], []

            def sc(i, qc, kb, kb_hi):
                delta = max(kb * P - qc * QW, 0)   # first valid t_q col
                sp = spsum.tile([P, 2, QW], F32, tag="s", name="s_ps")
                for h, ppos in ((0, (0, 0)), (1, (64, 0))):
                    nc.tensor.matmul(
                        sp[:, h, delta:QW],
                        qkT[h * 64:(h + 1) * 64, CT + hp,
                            kb * P:(kb + 1) * P],
                        qkT[h * 64:(h + 1) * 64, hp,
                            qc * QW + delta:(qc + 1) * QW],
                        start=True, stop=True, tile_position=ppos,
                    )
                s = scb.tile([P, 2, QW], BF16, tag="s")
                relu_evict(s[:, :, delta:QW], sp[:, :, delta:QW])
                if kb * P >= qc * QW:   # diagonal block: causal mask on the
                    # first P cols only (row p can only mask j' < p < P)
                    nc.gpsimd.affine_select(
                        s[:, :, delta:delta + P],
                        s[:, :, delta:delta + P],
                        pattern=[[0, 2], [1, P]],
                        compare_op=ALU.is_ge, fill=0.0,
                        base=0, channel_multiplier=-1,
                    )
                state["s"][i] = s

            def av(i, qc, kb, kb_hi):
                if kb == 0:
                    state["y"][qc] = ypsum.tile([P, QW], F32, tag="y",
                                                name="yp")
                yp = state["y"][qc]
                delta = max(kb * P - qc * QW, 0)
                s = state["s"].pop(i)
                # two heads accumulate into disjoint partition ranges of one
                # bank; each runs its own start/stop group
                nc.tensor.matmul(
                    yp[0:64, delta:QW], vsb[:, kb, hp * P:hp * P + 64],
                    s[:, 0, delta:QW],
                    start=(kb == 0), stop=(kb == kb_hi),
                    tile_position=(0, 0), skip_group_check=True,
                )
                nc.tensor.matmul(
                    yp[64:128, delta:QW],
                    vsb[:, kb, hp * P + 64:hp * P + 128],
                    s[:, 1, delta:QW],
                    start=(kb == 0), stop=(kb == kb_hi),
                    tile_position=(0, 64), skip_group_check=True,
                )
                if kb == kb_hi:
                    yp = state["y"].pop(qc)
                    yt = ysb.tile([P, QW], F32, tag="yt")
                    nc.vector.tensor_copy(yt[:], yp[:])
                    nc.sync.dma_start(
                        yT[hp * P:(hp + 1) * P, qc * QW:(qc + 1) * QW],
                        yt[:])

            for i, (qc, kb, kb_hi) in enumerate(items):
                sc_fns.append(
                    lambda i=i, qc=qc, kb=kb, kb_hi=kb_hi: sc(i, qc, kb, kb_hi))
                av_fns.append(
                    lambda i=i, qc=qc, kb=kb, kb_hi=kb_hi: av(i, qc, kb, kb_hi))
            return sc_fns, av_fns

        def group_fillers(g):
            """Fillers for group g's span: g's own v slices (front-loaded —
            consumed by g's lagged att@v steps) and qk for group g+1
            (paced evenly over the remaining steps)."""
            vq = [lambda s=g, tp=tp: emit_v(s, tp) for tp in range(TT // 2)]
            kq = []
            if g + 1 < NG:
                for hp in (2 * (g + 1), 2 * (g + 1) + 1):
                    kq.append(lambda ot=hp: emit_qk(ot))
                    kq.append(lambda ot=CT + hp: emit_qk(ot))
            return deque(vq), deque(kq)

        # ---- fused emission ----
        # pre-phase: group 0's q/k projections only (q first: its W block
        # leads the sync ring, k's leads the scalar ring)
        for ot in (0, 1, CT, CT + 1):
            emit_qk(ot)

        # Cross-group pipeline: each group's tail att@v steps interleave
        # with the next group's first score steps so the PE never dips at
        # group boundaries (a dip re-throttles the HAM clock gate).
        LAG = 2
        pending = []     # av emission thunk-lists carried from prev group
        for g in range(NG):
            streams = [attention_closures(hp) for hp in (2 * g, 2 * g + 1)]
            vq, kq = group_fillers(g)
            n = len(streams[0][0])
            nk, done_k = len(kq), 0
            for i in range(n):
                for sc_fns, _ in streams:
                    sc_fns[i]()
                if i < LAG:
                    if pending:
                        for fn in pending.pop(0):
                            fn()
                else:
                    for _, av_fns in streams:
                        av_fns[i - LAG]()
                if vq:                      # one v slice per early step
                    vq.popleft()()
                # pace qk fillers evenly across the remaining steps
                want = (nk * (i + 1)) // n
                while done_k < want:
                    kq.popleft()()
                    done_k += 1
            pending = [[av_fns[j] for _, av_fns in streams]
                       for j in range(n - LAG, n)]
        for tail in pending:
            for fn in tail:
                fn()

    nc.compile()
    return nc


_CACHE = {}


def _get_module():
    if "nc" not in _CACHE:
        _CACHE["nc"] = build_module()
    return _CACHE["nc"]


def _prep_in_maps(x, W_attn, b_attn, T=1024, C=768, n_cores=8):
    bf = ml_dtypes.bfloat16
    CT = C // P
    WT = np.ascontiguousarray(W_attn.astype(np.float32).T)     # [C, 3C]
    # [C, 3C] -> [p, ct, 3C]
    W3 = WT.reshape(CT, P, 3 * C).transpose(1, 0, 2)
    # 9 blocks of 256 o-cols in use order [q01,k01,v0,q23,k23,v1,q45,k45,v2]
    blocks = []
    for gg in range(3):
        blocks.append(W3[:, :, gg * 256:(gg + 1) * 256])             # q
        blocks.append(W3[:, :, C + gg * 256:C + (gg + 1) * 256])     # k
        blocks.append(W3[:, :, 2 * C + gg * 256:2 * C + (gg + 1) * 256])  # v
    wd = np.ascontiguousarray(
        np.stack(blocks, axis=1)).astype(bf)                   # [p, 9, ct, 256]
    bqk = np.ascontiguousarray(
        b_attn[:2 * C].astype(np.float32).reshape(2 * CT, P).T)  # [P, 12]
    bv = b_attn[2 * C:].astype(np.float32).reshape(3, 256)
    bvr = np.ascontiguousarray(
        np.repeat(bv, 2, axis=0).reshape(1, 2 * C)).astype(bf)  # [1, 2C]
    in_maps = []
    for c in range(n_cores):
        xT_b = np.ascontiguousarray(x[c].astype(np.float32).T)  # [C, T]
        xdb = np.ascontiguousarray(
            xT_b.reshape(CT, P, T).transpose(1, 0, 2)).astype(bf)
        in_maps.append({"xd": xdb, "wd": wd, "bqk": bqk, "bvr": bvr})
    return in_maps


def run(x, W_attn, b_attn, trace=False):
    nc = _get_module()
    in_maps = _prep_in_maps(x, W_attn, b_attn)
    res = bass_utils.run_bass_kernel_spmd(
        nc, in_maps, core_ids=list(range(8)), trace=trace)
    y = np.stack([np.asarray(res.results[c]["yT"]).T for c in range(8)])
    return np.ascontiguousarray(y.astype(np.float32)), res


def kernel(x, W_attn, b_attn):
    y, _ = run(x, W_attn, b_attn, trace=False)
    return y
